# revision 35
# baseline (speedup 1.0000x reference)
"""KoLeo loss kernel for Trainium2, 8 NeuronCores.

Strategy (data-parallel brute-force 1-NN over L2-normalized rows):
  - Each core gets a row-PERMUTED copy of x with its own 1024 rows first, so
    the self-match diagonal always falls in columns 0..1023 (core-invariant
    program, as required by SPMD).
  - Phase A is organized per 512-row chunk: one batched read DMA, four
    Square+accum activations (ACT), one Sqrt (ACT) + one reciprocal (DVE),
    four multiply+fp16-cast ops (DVE), one batched write DMA, then six XBAR
    transposing DMAs split across the SP and ACT hardware DGE queues.
  - dots slab = xT_own_cols^T @ xT (fp16 matmul, f32 PSUM accumulate) into
    two-bank PSUM tiles; each [128, 2, 512] PSUM tile is row-max-reduced in
    one DVE op into per-chunk slots (diag chunks get a -2*I mask add first);
    a final 16-way reduce per row tile yields the row max m.
  - PE warm-up: zero-valued fp16 matmuls accumulate +0 into the first real
    PSUM group so the PE's HAM clock gate reaches 8/8 early.
  - pdist for normalized vectors: dist = sqrt(2 - 2*m); per-core partial
    loss = sum(log(dist + EPS)); host combines: loss = -(sum partials)/8192.
"""

import os
import sys

sys.path.insert(0, "/opt/trn_rl_repo")

import numpy as np

import concourse.bass as bass
import concourse.mybir as mybir
import concourse.tile as tile
from concourse import bacc
from concourse.bass_utils import run_bass_kernel_spmd

B = 8192
D = 768
NCORES = 8
RPC = B // NCORES  # 1024 rows per core
P = 128
KC = D // P  # 6 contraction chunks
CH = 512  # moving chunk width
NCH = B // CH  # 16 chunks
T = RPC // P  # 8 row tiles per core
EPS = 1e-8
NWARM = int(os.environ.get("K_NWARM", "48"))  # HAM warm-up matmuls

f32 = mybir.dt.float32
f16 = mybir.dt.float16
AF = mybir.ActivationFunctionType
ALU = mybir.AluOpType
AX = mybir.AxisListType


def _build_program():
    nc = bacc.Bacc("TRN2", target_bir_lowering=False, debug=False,
                   enable_asserts=True)
    x_in = nc.dram_tensor("xm", [B, D], f32, kind="ExternalInput").ap()
    # planes 0-3: -2*I diag masks at offsets 0..3; plane 4: zeros
    mask4_in = nc.dram_tensor("mask4", [P, 5, CH], f32, kind="ExternalInput").ap()
    consts_in = nc.dram_tensor("consts", [P, 3], f32, kind="ExternalInput").ap()
    wz_in = nc.dram_tensor("wz", [P, CH], f16, kind="ExternalInput").ap()
    out_t = nc.dram_tensor("partial", [1, 1], f32, kind="ExternalOutput").ap()

    with tile.TileContext(nc) as tc:
        with (
            tc.tile_pool(name="big", bufs=1) as big,
            tc.tile_pool(name="work", bufs=3) as work,
            tc.tile_pool(name="sqp", bufs=2) as sqp,
            tc.tile_pool(name="small", bufs=4) as small,
            tc.tile_pool(name="psum", bufs=3, space="PSUM") as psum_pool,
            tc.tile_pool(name="psum1", bufs=1, space="PSUM") as psum1_pool,
            tc.tile_pool(name="dram", bufs=1, space="DRAM") as dram_pool,
        ):
            # persistent tiles
            xTn = [big.tile([P, KC, CH], f16, name=f"xT{n}", tag=f"xT{n}")
                   for n in range(NCH)]
            mask4 = big.tile([P, 5, CH], f32, tag="mask4")
            consts = big.tile([P, 3], f32, tag="consts")
            wz = big.tile([P, CH], f16, tag="wz")
            rmax0 = big.tile([P, T], f32, tag="rmax0")
            rmaxall = big.tile([P, NCH, T], f32, tag="rmaxall")

            nc.sync.dma_start(mask4[:], mask4_in)
            nc.sync.dma_start(consts[:], consts_in)
            nc.sync.dma_start(wz[:], wz_in)
            ones = consts[:, 0:1]
            two = consts[:, 1:2]
            epsb = consts[:, 2:3]
            warm_l = wz[:, 0:P]

            xn_dram = dram_pool.tile([B, D], f16, name="xn_dram")

            # Phase A: per 512-row chunk.  Reads go on the ACT DGE queue
            # (they have no upstream deps, so they never block it); the
            # write and all transposes go on the SP queue, where same-queue
            # ordering makes the transposes start as soon as the write
            # lands without cross-queue stalls.
            for n in range(NCH):
                rt4 = work.tile([P, 4, D], f32, tag="rt4")
                nc.scalar.dma_start(
                    rt4[:],
                    x_in[n * CH : (n + 1) * CH, :].rearrange(
                        "(k p) d -> p k d", p=P
                    ),
                )
                sq4 = sqp.tile([P, 4, D], f32, tag="sq4")
                ss4 = small.tile([P, 4], f32, tag="ss4")
                for k in range(4):
                    nc.scalar.activation(
                        sq4[:, k, :], rt4[:, k, :], AF.Square,
                        accum_out=ss4[:, k : k + 1],
                    )
                nrm4 = small.tile([P, 4], f32, tag="nrm4")
                nc.scalar.activation(nrm4[:], ss4[:], AF.Sqrt)
                rinv4 = small.tile([P, 4], f32, tag="rinv4")
                nc.vector.reciprocal(rinv4[:], nrm4[:])
                xn4 = work.tile([P, 4, D], f16, tag="xn4")
                for k in range(4):
                    nc.vector.tensor_scalar_mul(
                        xn4[:, k, :], rt4[:, k, :], rinv4[:, k : k + 1]
                    )
                nc.sync.dma_start(
                    xn_dram[n * CH : (n + 1) * CH, :].rearrange(
                        "(k p) d -> p k d", p=P
                    ),
                    xn4[:],
                )
                for c in range(KC):
                    nc.sync.dma_start_transpose(
                        xTn[n][:, c, :],
                        xn_dram[n * CH : (n + 1) * CH, c * P : (c + 1) * P],
                    )

            # Phase B: matmul + per-pair row max into slots
            for n in range(NCH):
                for tp in range(T // 2):  # pairs of row tiles
                    pt2 = psum_pool.tile([P, 2, CH], f32, tag="pt2")
                    for half in range(2):
                        t = tp * 2 + half
                        if n == 0 and t == 0:
                            for w in range(NWARM):
                                nc.tensor.matmul(
                                    pt2[:, 0, :], lhsT=warm_l, rhs=wz[:],
                                    start=(w == 0), stop=False,
                                )
                        for c in range(KC):
                            nt = t // 4
                            nc.tensor.matmul(
                                pt2[:, half, :],
                                lhsT=xTn[nt][:, c,
                                             (t % 4) * P : (t % 4 + 1) * P],
                                rhs=xTn[n][:, c, :],
                                start=(c == 0 and not (n == 0 and t == 0)),
                                stop=(c == KC - 1),
                            )
                    if n == tp // 2:  # diag chunk for both tiles of the pair
                        for half in range(2):
                            t = tp * 2 + half
                            nc.vector.tensor_tensor(
                                out=pt2[:, half, :], in0=pt2[:, half, :],
                                in1=mask4[:, t % 4], op=ALU.add,
                            )
                    nc.vector.tensor_reduce(
                        rmaxall[:, n, 2 * tp : 2 * tp + 2], pt2[:],
                        axis=AX.X, op=ALU.max,
                    )

            # Phase C: merge slots -> dist -> log -> partial sum
            for t in range(T):
                nc.vector.tensor_reduce(
                    rmax0[:, t : t + 1], rmaxall[:, :, t], axis=AX.X,
                    op=ALU.max,
                )
            dist = big.tile([P, T], f32, tag="dist")
            nc.scalar.activation(dist[:], rmax0[:], AF.Sqrt, scale=-2.0,
                                 bias=two)
            logd = big.tile([P, T], f32, tag="logd")
            lsum = big.tile([P, 1], f32, tag="lsum")
            nc.scalar.activation(
                logd[:], dist[:], AF.Ln, bias=epsb, accum_out=lsum[:]
            )
            pfin = psum1_pool.tile([1, 1], f32, tag="pfin")
            nc.tensor.matmul(pfin[:], lhsT=ones, rhs=lsum[:], start=True,
                             stop=True)
            res = big.tile([1, 1], f32, tag="res")
            nc.vector.tensor_copy(out=res[:], in_=pfin[:])
            nc.sync.dma_start(out_t[:], res[:])

    nc.compile()
    return nc


_NC_CACHE = None


def _get_nc():
    global _NC_CACHE
    if _NC_CACHE is None:
        _NC_CACHE = _build_program()
    return _NC_CACHE


def _make_in_maps(x: np.ndarray):
    mask4 = np.zeros((P, 5, CH), dtype=np.float32)
    for v in range(4):
        mask4[:, v, v * P : (v + 1) * P] = -2.0 * np.eye(P, dtype=np.float32)
    consts = np.zeros((P, 3), dtype=np.float32)
    consts[:, 0] = 1.0
    consts[:, 1] = 2.0
    consts[:, 2] = EPS
    wz = np.zeros((P, CH), dtype=np.float16)
    in_maps = []
    for m in range(NCORES):
        own = x[m * RPC : (m + 1) * RPC]
        rest = np.concatenate([x[: m * RPC], x[(m + 1) * RPC :]], axis=0)
        xm = np.ascontiguousarray(np.concatenate([own, rest], axis=0))
        in_maps.append({"xm": xm, "mask4": mask4, "consts": consts, "wz": wz})
    return in_maps


def kernel(student_output: np.ndarray) -> np.ndarray:
    x = np.asarray(student_output, dtype=np.float32)
    nc = _get_nc()
    in_maps = _make_in_maps(x)
    res = run_bass_kernel_spmd(nc, in_maps, list(range(NCORES)))
    total = 0.0
    for r in res.results:
        total += float(r["partial"].reshape(()))
    loss = -(total / B)
    return np.float32(loss)


# revision 36
# speedup vs baseline: 1.2128x; 1.2128x over previous
"""KoLeo loss kernel for Trainium2, 8 NeuronCores — fp8 DoubleRow edition.

Strategy (data-parallel brute-force 1-NN over L2-normalized rows):
  - Each core gets a row-PERMUTED copy of x with its own 1024 rows first, so
    the self-match diagonal always falls in columns 0..1023 (core-invariant
    program, as required by SPMD).
  - Phase A per 512-row chunk: one batched read (ACT DGE queue), four
    Square+accum activations (ACT), one scaled Sqrt + one reciprocal giving
    64/||x||, four multiply+fp8e4-cast ops (DVE, values scaled by 64 to sit
    in e4m3's sweet spot), one batched fp8 write + three u16-pair XBAR
    transposing DMAs (SP DGE queue; same-queue ordering avoids stalls).
  - The transposed tiles hold fp8 pairs (dims 2q, 2q+1) per partition as the
    two bytes of each u16 — exactly the layout DoubleRow's moving operand
    wants ([p, s, j] with 1-byte s-stride).  Weights are deinterleaved into
    plane layout ([p, s, j], 128-byte s-stride) by 24 small DVE copies.
  - dots slab: 3 DoubleRow fp8 matmuls per (row-tile, chunk) — contraction
    256 per MM — accumulating into two-bank PSUM tiles; each [128, 2, 512]
    PSUM tile is row-max-reduced in one DVE op into per-chunk slots (diag
    chunks get a -2*4096*I mask add first); a 16-way reduce per row tile
    yields the scaled row max m4096.
  - PE warm-up: zero-valued fp8 matmuls accumulate +0 into the first group.
  - dist = sqrt(2 - 2*m4096/4096); per-core partial = sum(log(dist + EPS));
    host: loss = -(sum partials)/8192.
"""

import os
import sys

sys.path.insert(0, "/opt/trn_rl_repo")

import numpy as np

import concourse.bass as bass
import concourse.mybir as mybir
import concourse.tile as tile
from concourse import bacc
from concourse.bass_utils import run_bass_kernel_spmd

B = 8192
D = 768
NCORES = 8
RPC = B // NCORES  # 1024 rows per core
P = 128
CP = 3  # contraction pair-chunks (each covers 256 dims)
CH = 512  # moving chunk width
NCH = B // CH  # 16 chunks
T = RPC // P  # 8 row tiles per core
EPS = 1e-8
SCALE = 64.0  # fp8 domain scale; dots come out scaled by SCALE**2 = 4096
NWARM = int(os.environ.get("K_NWARM", "48"))  # HAM warm-up matmuls

f32 = mybir.dt.float32
f8 = mybir.dt.float8e4
u16 = mybir.dt.uint16
AF = mybir.ActivationFunctionType
ALU = mybir.AluOpType
AX = mybir.AxisListType
DR = mybir.MatmulPerfMode.DoubleRow


def _build_program():
    nc = bacc.Bacc("TRN2", target_bir_lowering=False, debug=False,
                   enable_asserts=True)
    x_in = nc.dram_tensor("xm", [B, D], f32, kind="ExternalInput").ap()
    # planes 0-3: -2*4096*I diag masks at offsets 0..3; plane 4: zeros
    mask4_in = nc.dram_tensor("mask4", [P, 5, CH], f32, kind="ExternalInput").ap()
    consts_in = nc.dram_tensor("consts", [P, 3], f32, kind="ExternalInput").ap()
    wz_in = nc.dram_tensor("wz", [P, 2 * CH], f8, kind="ExternalInput").ap()
    out_t = nc.dram_tensor("partial", [1, 1], f32, kind="ExternalOutput").ap()

    with tile.TileContext(nc) as tc:
        with (
            tc.tile_pool(name="big", bufs=1) as big,
            tc.tile_pool(name="work", bufs=3) as work,
            tc.tile_pool(name="sqp", bufs=2) as sqp,
            tc.tile_pool(name="small", bufs=4) as small,
            tc.tile_pool(name="psum", bufs=3, space="PSUM") as psum_pool,
            tc.tile_pool(name="psum1", bufs=1, space="PSUM") as psum1_pool,
            tc.tile_pool(name="dram", bufs=1, space="DRAM") as dram_pool,
        ):
            # persistent tiles
            xT8 = [big.tile([P, CP, CH], u16, name=f"xT{n}", tag=f"xT{n}")
                   for n in range(NCH)]
            wpl = [big.tile([P, CP, 2, P], f8, name=f"wp{t}", tag=f"wp{t}")
                   for t in range(T)]
            mask4 = big.tile([P, 5, CH], f32, tag="mask4")
            consts = big.tile([P, 3], f32, tag="consts")
            wz = big.tile([P, 2 * CH], f8, tag="wz")
            rmax0 = big.tile([P, T], f32, tag="rmax0")
            rmaxall = big.tile([P, NCH, T], f32, tag="rmaxall")

            nc.sync.dma_start(mask4[:], mask4_in)
            nc.sync.dma_start(consts[:], consts_in)
            nc.sync.dma_start(wz[:], wz_in)
            ones = consts[:, 0:1]
            two = consts[:, 1:2]
            epsb = consts[:, 2:3]
            warm_l = wz[:, 0 : 2 * P].rearrange("p (s j) -> p s j", s=2)
            warm_r = wz[:].rearrange("p (s j) -> p s j", s=2)

            xn_dram = dram_pool.tile([B, D // 2], u16, name="xn_dram")

            # Phase A: per 512-row chunk
            for n in range(NCH):
                rt4 = work.tile([P, 4, D], f32, tag="rt4")
                nc.scalar.dma_start(
                    rt4[:],
                    x_in[n * CH : (n + 1) * CH, :].rearrange(
                        "(k p) d -> p k d", p=P
                    ),
                )
                sq4 = sqp.tile([P, 4, D], f32, tag="sq4")
                ss4 = small.tile([P, 4], f32, tag="ss4")
                for k in range(4):
                    nc.scalar.activation(
                        sq4[:, k, :], rt4[:, k, :], AF.Square,
                        accum_out=ss4[:, k : k + 1],
                    )
                # nrm64 = ||x||/64; rinv64 = 64/||x||
                nrm4 = small.tile([P, 4], f32, tag="nrm4")
                nc.scalar.activation(nrm4[:], ss4[:], AF.Sqrt,
                                     scale=1.0 / (SCALE * SCALE))
                rinv4 = small.tile([P, 4], f32, tag="rinv4")
                nc.vector.reciprocal(rinv4[:], nrm4[:])
                xn4 = work.tile([P, 4, D], f8, tag="xn4")
                for k in range(4):
                    nc.vector.tensor_scalar_mul(
                        xn4[:, k, :], rt4[:, k, :], rinv4[:, k : k + 1]
                    )
                nc.sync.dma_start(
                    xn_dram[n * CH : (n + 1) * CH, :].bitcast(f8).rearrange(
                        "(k p) d -> p k d", p=P
                    ),
                    xn4[:],
                )
                for c in range(CP):
                    nc.sync.dma_start_transpose(
                        xT8[n][:, c, :],
                        xn_dram[n * CH : (n + 1) * CH, c * P : (c + 1) * P],
                    )
                if n <= 1:
                    # deinterleave own-row weights into plane layout
                    for tt in range(4 * n, 4 * n + 4):
                        for c in range(CP):
                            src = xT8[n][:, c,
                                         (tt % 4) * P : (tt % 4 + 1) * P]
                            nc.vector.tensor_copy(
                                out=wpl[tt][:, c, :, :],
                                in_=src.bitcast(f8).rearrange(
                                    "p (j s) -> p s j", s=2
                                ),
                            )

            # Phase B: DoubleRow matmuls + per-pair row max into slots
            for n in range(NCH):
                for tp in range(T // 2):  # pairs of row tiles
                    pt2 = psum_pool.tile([P, 2, CH], f32, tag="pt2")
                    for half in range(2):
                        t = tp * 2 + half
                        if n == 0 and t == 0:
                            for w in range(NWARM):
                                nc.tensor.matmul(
                                    pt2[:, 0, :], lhsT=warm_l, rhs=warm_r,
                                    start=(w == 0), stop=False,
                                    perf_mode=DR,
                                )
                        rhs_n = xT8[n][:].bitcast(f8)
                        for c in range(CP):
                            nc.tensor.matmul(
                                pt2[:, half, :],
                                lhsT=wpl[t][:, c, :, :],
                                rhs=rhs_n[:, c, :].rearrange(
                                    "p (j s) -> p s j", s=2
                                ),
                                start=(c == 0 and not (n == 0 and t == 0)),
                                stop=(c == CP - 1),
                                perf_mode=DR,
                            )
                    if n == tp // 2:  # diag chunk for both tiles of the pair
                        for half in range(2):
                            t = tp * 2 + half
                            nc.vector.tensor_tensor(
                                out=pt2[:, half, :], in0=pt2[:, half, :],
                                in1=mask4[:, t % 4], op=ALU.add,
                            )
                    nc.vector.tensor_reduce(
                        rmaxall[:, n, 2 * tp : 2 * tp + 2], pt2[:],
                        axis=AX.X, op=ALU.max,
                    )

            # Phase C: merge slots -> dist -> log -> partial sum
            for t in range(T):
                nc.vector.tensor_reduce(
                    rmax0[:, t : t + 1], rmaxall[:, :, t], axis=AX.X,
                    op=ALU.max,
                )
            dist = big.tile([P, T], f32, tag="dist")
            nc.scalar.activation(dist[:], rmax0[:], AF.Sqrt,
                                 scale=-2.0 / (SCALE * SCALE), bias=two)
            logd = big.tile([P, T], f32, tag="logd")
            lsum = big.tile([P, 1], f32, tag="lsum")
            nc.scalar.activation(
                logd[:], dist[:], AF.Ln, bias=epsb, accum_out=lsum[:]
            )
            pfin = psum1_pool.tile([1, 1], f32, tag="pfin")
            nc.tensor.matmul(pfin[:], lhsT=ones, rhs=lsum[:], start=True,
                             stop=True)
            res = big.tile([1, 1], f32, tag="res")
            nc.vector.tensor_copy(out=res[:], in_=pfin[:])
            nc.sync.dma_start(out_t[:], res[:])

    nc.compile()
    return nc


_NC_CACHE = None


def _get_nc():
    global _NC_CACHE
    if _NC_CACHE is None:
        _NC_CACHE = _build_program()
    return _NC_CACHE


def _make_in_maps(x: np.ndarray):
    import ml_dtypes

    mask4 = np.zeros((P, 5, CH), dtype=np.float32)
    for v in range(4):
        mask4[:, v, v * P : (v + 1) * P] = (
            -2.0 * SCALE * SCALE * np.eye(P, dtype=np.float32)
        )
    consts = np.zeros((P, 3), dtype=np.float32)
    consts[:, 0] = 1.0
    consts[:, 1] = 2.0
    consts[:, 2] = EPS
    wz = np.zeros((P, 2 * CH), dtype=ml_dtypes.float8_e4m3)
    in_maps = []
    for m in range(NCORES):
        own = x[m * RPC : (m + 1) * RPC]
        rest = np.concatenate([x[: m * RPC], x[(m + 1) * RPC :]], axis=0)
        xm = np.ascontiguousarray(np.concatenate([own, rest], axis=0))
        in_maps.append({"xm": xm, "mask4": mask4, "consts": consts, "wz": wz})
    return in_maps


def kernel(student_output: np.ndarray) -> np.ndarray:
    x = np.asarray(student_output, dtype=np.float32)
    nc = _get_nc()
    in_maps = _make_in_maps(x)
    res = run_bass_kernel_spmd(nc, in_maps, list(range(NCORES)))
    total = 0.0
    for r in res.results:
        total += float(r["partial"].reshape(()))
    loss = -(total / B)
    return np.float32(loss)
